# revision 17
# baseline (speedup 1.0000x reference)
"""Trainium2 Bass kernel for nn_Slots: out[b,s,d] = sum_hw feats[b,d,hw] * masks[s,hw].

Data-parallel over B across 8 cores (32 batches/core). The kernel is a pure
DMA-roofline pipeline: feats are staged host-side in hw-major layout
(featsT[b] = feats[b].T, shape (784, 512)) so each batch is one contiguous
2048B-descriptor load straight into the matmul operand layout — no on-device
transposes at all.

Per batch b:
  - SWDGE load featsT[b] -> ft tile [112, 7*512] (f32r; 7 hw-chunks of 112)
  - 7 accumulating PE matmuls po[126,512] += mk[:,c].T @ ft[:,c] (K=112,
    stationary masksT chunk, f32r moving operand -> 1 cyc/row)
  - ACT copy po (PSUM) -> ot (SBUF; fp16 downcast, one tile per 4 batches)
  - HWDGE store ot -> out[4g:4g+4], triggered from the SP queue

The output leaves the device as fp16 (upcast to f32 on the host): fp16
rounding adds ~5e-4 relative error on top of the ~1.6e-4 from f32r matmuls,
far inside the 2e-2 gate, and halves the store traffic.

The DMA engines are the bottleneck (59.7 MB through 360 GB/s = 166.8 us), so
the schedule packs them gaplessly: all 32 loads run back-to-back on the
Pool/SWDGE queue (ft/po rotate over 4 tags; compute trails by ~1 batch), and
ALL stores are held back — the first SP store trigger carries an extra dep on
load 29's completion — so the 32 stores pack back-to-back right after the
last load. The last store's compute chain (mm31 -> copy -> trigger prep) is
fully hidden under the 31 earlier stores, leaving only lead-in + sem
propagation + drain (~3.8 us) over the DMA busy floor.

masksT is prearranged host-side into the exact SBUF tile layout
(112, 7*126) and loaded with a single contiguous DMA on the SP queue.
float32r is bit-identical to float32, so all DRAM tensors are declared f32r
and fed plain f32 numpy arrays; matmuls then run at 1 cycle/row.
"""

import numpy as np
from contextlib import ExitStack

import concourse.bass as bass
import concourse.tile as tile
from concourse import mybir
from concourse.bass_utils import run_bass_kernel_spmd
from concourse.tile_rust import add_dep_helper

N_CORES = 8
B_FULL, D, H, W = 256, 512, 28, 28
HW = H * W           # 784
S = 126
B_LOC = B_FULL // N_CORES  # 32
KC = 112             # hw contraction chunk (7 * 112 = 784)
NCHUNK = HW // KC    # 7

F32 = mybir.dt.float32
F32R = mybir.dt.float32r
F16 = mybir.dt.float16

NBUF = 4             # rotation depth for ft/po tiles
SB = 4               # batches per store DMA
HOLD = 29            # stores wait for this load before transferring

_CACHE = {}
SPLIT_DRAIN = True  # set False for CoreSim (it rejects post-scheduler NoOps)


def _build_program(reps=1):
    nc = bass.Bass("TRN2", target_bir_lowering=False, debug=False)
    featsT = nc.dram_tensor("featsT", (B_LOC, HW, D), F32R,
                            kind="ExternalInput").ap()
    masksL = nc.dram_tensor("masksL", (KC, NCHUNK * S), F32R,
                            kind="ExternalInput").ap()
    out = nc.dram_tensor("out", (B_LOC, S, D), F16, kind="ExternalOutput").ap()

    with ExitStack() as ctx:
        tc = ctx.enter_context(tile.TileContext(nc))
        const_pool = ctx.enter_context(tc.tile_pool(name="const", bufs=1))
        ft_pool = ctx.enter_context(tc.tile_pool(name="ftp", bufs=1))
        ot_pool = ctx.enter_context(tc.tile_pool(name="otp", bufs=1))
        po_pool = ctx.enter_context(tc.tile_pool(name="pop", bufs=1, space="PSUM"))

        def order(later, earlier):
            add_dep_helper(later.ins, earlier.ins, sync=False, reason="order")

        mk = const_pool.tile([KC, NCHUNK * S], F32R, name="mk")
        mk_dma = nc.sync.dma_start(
            mk.rearrange("p (c s) -> p c s", s=S),
            masksL.rearrange("p (c s) -> p c s", s=S),
        )

        prev_pool = None
        prev_pe = None
        prev_act = None
        prev_sp = mk_dma
        dma_ins = []
        hold_deps = []

        for gb in range(reps * B_LOC):
            b = gb % B_LOC
            ft = ft_pool.tile([KC, NCHUNK * D], F32R, name="ft",
                              tag=f"ft{gb % NBUF}", bufs=1)
            dma_in = nc.gpsimd.dma_start(
                ft.rearrange("p (c d) -> p c d", d=D),
                featsT[b].rearrange("(c p) d -> p c d", p=KC),
            )
            if prev_pool is not None:
                order(dma_in, prev_pool)
            prev_pool = dma_in
            dma_ins.append(dma_in)

            po = po_pool.tile([S, D], F32, name="po", tag=f"po{gb % NBUF}",
                              bufs=1)
            for c in range(NCHUNK):
                mm = nc.tensor.matmul(
                    po[:], mk[:, c * S:(c + 1) * S],
                    ft[:, c * D:(c + 1) * D],
                    start=(c == 0), stop=(c == NCHUNK - 1),
                )
                if prev_pe is not None:
                    order(mm, prev_pe)
                prev_pe = mm

            g, j = divmod(b, SB)
            if j == 0:
                ot = ot_pool.tile([S, SB * D], F16, name="ot",
                                  tag=f"ot{(gb // SB) % (B_LOC // SB)}",
                                  bufs=1)
                ots = ot
            cp = nc.scalar.activation(ots[:, j * D:(j + 1) * D], po[:],
                                      mybir.ActivationFunctionType.Copy)
            if prev_act is not None:
                order(cp, prev_act)
            prev_act = cp
            if j == SB - 1:
                dma_out = nc.sync.dma_start(
                    out[g * SB:(g + 1) * SB].rearrange("j s d -> s j d"),
                    ots.rearrange("s (j d) -> s j d", d=D),
                )
                order(dma_out, prev_sp)
                if g == 0:
                    hold_deps.append(dma_out)
                prev_sp = dma_out

        # Hold each rep's stores back behind that rep's load HOLD so the
        # store burst packs gaplessly right after the last load (SP queue is
        # in-order, so gating store 0 gates them all).
        for r, hd in enumerate(hold_deps):
            add_dep_helper(hd.ins, dma_ins[r * B_LOC + HOLD].ins,
                           sync=True, reason="store holdback")

    if SPLIT_DRAIN:
        _split_drain_waits(nc)
    return nc


def _split_drain_waits(nc, max_waits=1):
    """TRN2 queue instructions support one sync wait. Anything the scheduler
    left with more gets its excess waits moved onto single-wait NoOps
    inserted right before it on the same engine queue (in-order, so the
    semantics are identical)."""
    for f in nc.m.functions:
        for blk in getattr(f, "blocks", []):
            insts = blk.instructions
            i = 0
            while i < len(insts):
                inst = insts[i]
                si = getattr(inst, "sync_info", None)
                if (si is not None and len(si.on_wait) > max_waits):
                    waits = list(si.on_wait)
                    keep = waits[-max_waits:]
                    move = waits[:-max_waits]
                    for k, w in enumerate(move):
                        nop = mybir.InstNoOp(
                            name=f"{inst.name}-ws{k}",
                            engine=inst.engine,
                            bass_nofuse=True,
                            sync_info=mybir.SyncInfo(on_wait=[w], on_update=[]),
                        )
                        insts.insert(i, nop)
                        i += 1
                    si.on_wait = keep
                i += 1


def get_program(reps=1):
    key = "nc" if reps == 1 else f"nc{reps}"
    if key not in _CACHE:
        _CACHE[key] = _build_program(reps)
    return _CACHE[key]


def make_in_maps(feats, masks):
    feats = np.ascontiguousarray(np.asarray(feats, dtype=np.float32))
    masks = np.asarray(masks, dtype=np.float32)
    # masksL[p, c*S + s] = masks[s, c*KC + p]
    masksL = np.ascontiguousarray(
        masks.reshape(S, HW).T.reshape(NCHUNK, KC, S)
        .transpose(1, 0, 2).reshape(KC, NCHUNK * S))
    fr = feats.reshape(N_CORES, B_LOC, D, HW)
    return [
        {
            "featsT": np.ascontiguousarray(fr[i].transpose(0, 2, 1)),
            "masksL": masksL,
        }
        for i in range(N_CORES)
    ]


def kernel(feats, masks, _trace=False, _tmpdir=None):
    nc = get_program()
    in_maps = make_in_maps(feats, masks)
    res = run_bass_kernel_spmd(
        nc, in_maps, core_ids=list(range(N_CORES)),
        trace=_trace, tmpdir=_tmpdir,
    )
    out = np.concatenate([r["out"] for r in res.results], axis=0)
    out = out.astype(np.float32)
    if _trace:
        _CACHE["last_results"] = res
    return out


# revision 18
# speedup vs baseline: 1.0022x; 1.0022x over previous
"""Trainium2 Bass kernel for nn_Slots: out[b,s,d] = sum_hw feats[b,d,hw] * masks[s,hw].

Data-parallel over B across 8 cores (32 batches/core). The kernel is a pure
DMA-roofline pipeline: feats are staged host-side in hw-major layout
(featsT[b] = feats[b].T, shape (784, 512)) so each batch is one contiguous
2048B-descriptor load straight into the matmul operand layout — no on-device
transposes at all.

Per batch b:
  - SWDGE load featsT[b] -> ft tile [112, 7*512] (f32r; 7 hw-chunks of 112)
  - 7 accumulating PE matmuls po[126,512] += mk[:,c].T @ ft[:,c] (K=112,
    stationary masksT chunk, f32r moving operand -> 1 cyc/row)
  - ACT copy po (PSUM) -> ot (SBUF; fp16 downcast, one tile per 4 batches)
  - HWDGE store ot -> out[4g:4g+4], triggered from the SP queue

The output leaves the device as fp16 (upcast to f32 on the host): fp16
rounding adds ~5e-4 relative error on top of the ~1.6e-4 from f32r matmuls,
far inside the 2e-2 gate, and halves the store traffic.

The DMA engines are the bottleneck (59.7 MB through 360 GB/s = 166.8 us), so
the schedule packs them gaplessly: all 32 loads run back-to-back on the
Pool/SWDGE queue (ft/po rotate over 4 tags; compute trails by ~1 batch), and
ALL stores are held back — the first SP store trigger carries an extra dep on
load 29's completion — so the 32 stores pack back-to-back right after the
last load. The last store's compute chain (mm31 -> copy -> trigger prep) is
fully hidden under the 31 earlier stores, leaving only lead-in + sem
propagation + drain (~3.8 us) over the DMA busy floor.

masksT is prearranged host-side into the exact SBUF tile layout
(112, 7*126) and loaded with a single contiguous DMA on the SP queue.
float32r is bit-identical to float32, so all DRAM tensors are declared f32r
and fed plain f32 numpy arrays; matmuls then run at 1 cycle/row.
"""

import numpy as np
from contextlib import ExitStack

import concourse.bass as bass
import concourse.tile as tile
from concourse import mybir
from concourse.bass_utils import run_bass_kernel_spmd
from concourse.tile_rust import add_dep_helper

N_CORES = 8
B_FULL, D, H, W = 256, 512, 28, 28
HW = H * W           # 784
S = 126
B_LOC = B_FULL // N_CORES  # 32
KC = 112             # hw contraction chunk (7 * 112 = 784)
NCHUNK = HW // KC    # 7

F32 = mybir.dt.float32
F32R = mybir.dt.float32r
F16 = mybir.dt.float16

NBUF = 4             # rotation depth for ft/po tiles
SB = 4               # batches per store DMA
HOLD = 29            # stores wait for this load before transferring

_CACHE = {}
SPLIT_DRAIN = True  # set False for CoreSim (it rejects post-scheduler NoOps)


def _build_program(reps=1):
    nc = bass.Bass("TRN2", target_bir_lowering=False, debug=False)
    featsT = nc.dram_tensor("featsT", (B_LOC, HW, D), F32R,
                            kind="ExternalInput").ap()
    masksL = nc.dram_tensor("masksL", (KC, NCHUNK * S), F32R,
                            kind="ExternalInput").ap()
    out = nc.dram_tensor("out", (B_LOC, S, D), F16, kind="ExternalOutput").ap()

    with ExitStack() as ctx:
        tc = ctx.enter_context(tile.TileContext(nc))
        const_pool = ctx.enter_context(tc.tile_pool(name="const", bufs=1))
        ft_pool = ctx.enter_context(tc.tile_pool(name="ftp", bufs=1))
        ot_pool = ctx.enter_context(tc.tile_pool(name="otp", bufs=1))
        po_pool = ctx.enter_context(tc.tile_pool(name="pop", bufs=1, space="PSUM"))

        def order(later, earlier):
            add_dep_helper(later.ins, earlier.ins, sync=False, reason="order")

        mk = const_pool.tile([KC, NCHUNK * S], F32R, name="mk")
        mk_dma = nc.sync.dma_start(
            mk.rearrange("p (c s) -> p c s", s=S),
            masksL.rearrange("p (c s) -> p c s", s=S),
        )

        prev_pool = None
        prev_pe = None
        prev_act = None
        prev_sp = mk_dma
        dma_ins = []
        hold_deps = []

        for gb in range(reps * B_LOC):
            b = gb % B_LOC
            ft = ft_pool.tile([KC, NCHUNK * D], F32R, name="ft",
                              tag=f"ft{gb % NBUF}", bufs=1)
            dma_in = nc.gpsimd.dma_start(
                ft.rearrange("p (c d) -> p c d", d=D),
                featsT[b].rearrange("(c p) d -> p c d", p=KC),
            )
            if prev_pool is not None:
                order(dma_in, prev_pool)
            prev_pool = dma_in
            dma_ins.append(dma_in)

            po = po_pool.tile([S, D], F32, name="po", tag=f"po{gb % NBUF}",
                              bufs=1)
            for c in range(NCHUNK):
                mm = nc.tensor.matmul(
                    po[:], mk[:, c * S:(c + 1) * S],
                    ft[:, c * D:(c + 1) * D],
                    start=(c == 0), stop=(c == NCHUNK - 1),
                )
                if prev_pe is not None:
                    order(mm, prev_pe)
                prev_pe = mm

            g, j = divmod(b, SB)
            if j == 0:
                ot = ot_pool.tile([S, SB * D], F16, name="ot",
                                  tag=f"ot{(gb // SB) % (B_LOC // SB)}",
                                  bufs=1)
                ots = ot
            cp = nc.scalar.activation(ots[:, j * D:(j + 1) * D], po[:],
                                      mybir.ActivationFunctionType.Copy)
            if prev_act is not None:
                order(cp, prev_act)
            prev_act = cp
            if j == SB - 1:
                dma_out = nc.sync.dma_start(
                    out[g * SB:(g + 1) * SB].rearrange("j s d -> s j d"),
                    ots.rearrange("s (j d) -> s j d", d=D),
                )
                order(dma_out, prev_sp)
                if g == 0:
                    hold_deps.append(dma_out)
                prev_sp = dma_out

        # Hold each rep's stores back behind that rep's load HOLD so the
        # store burst packs gaplessly right after the last load (SP queue is
        # in-order, so gating store 0 gates them all).
        for r, hd in enumerate(hold_deps):
            add_dep_helper(hd.ins, dma_ins[r * B_LOC + HOLD].ins,
                           sync=True, reason="store holdback")

    if SPLIT_DRAIN:
        _split_drain_waits(nc)
    return nc


def _wait_order_key(w):
    """Static estimate of sem firing order for this program: load (SWDGE)
    sems fire first, then PE / ACT compute sems, then store (HWDGE) sems —
    within HWDGE, the lane with the higher wait value fires last (it is
    reused by the final store). Puts the latest-firing wait on the real
    instruction so satisfied-NoOp decode never trails the last semaphore."""
    name = w.ant_name or ""
    if name.startswith("DMASW"):
        cls = 0
    elif name.startswith("PE"):
        cls = 1
    elif name.startswith("Activation"):
        cls = 2
    elif name.startswith("DMAHW"):
        cls = 3
    else:
        cls = 1
    return (cls, w.wait_value if w.wait_value is not None else 0)


def _split_drain_waits(nc, max_waits=1):
    """TRN2 queue instructions support one sync wait. Anything the scheduler
    left with more gets its excess waits moved onto single-wait NoOps
    inserted right before it on the same engine queue (in-order, so the
    semantics are identical)."""
    for f in nc.m.functions:
        for blk in getattr(f, "blocks", []):
            insts = blk.instructions
            i = 0
            while i < len(insts):
                inst = insts[i]
                si = getattr(inst, "sync_info", None)
                if (si is not None and len(si.on_wait) > max_waits):
                    waits = sorted(si.on_wait, key=_wait_order_key)
                    keep = waits[-max_waits:]
                    move = waits[:-max_waits]
                    for k, w in enumerate(move):
                        nop = mybir.InstNoOp(
                            name=f"{inst.name}-ws{k}",
                            engine=inst.engine,
                            bass_nofuse=True,
                            sync_info=mybir.SyncInfo(on_wait=[w], on_update=[]),
                        )
                        insts.insert(i, nop)
                        i += 1
                    si.on_wait = keep
                i += 1


def get_program(reps=1):
    key = "nc" if reps == 1 else f"nc{reps}"
    if key not in _CACHE:
        _CACHE[key] = _build_program(reps)
    return _CACHE[key]


def make_in_maps(feats, masks):
    feats = np.ascontiguousarray(np.asarray(feats, dtype=np.float32))
    masks = np.asarray(masks, dtype=np.float32)
    # masksL[p, c*S + s] = masks[s, c*KC + p]
    masksL = np.ascontiguousarray(
        masks.reshape(S, HW).T.reshape(NCHUNK, KC, S)
        .transpose(1, 0, 2).reshape(KC, NCHUNK * S))
    fr = feats.reshape(N_CORES, B_LOC, D, HW)
    return [
        {
            "featsT": np.ascontiguousarray(fr[i].transpose(0, 2, 1)),
            "masksL": masksL,
        }
        for i in range(N_CORES)
    ]


def kernel(feats, masks, _trace=False, _tmpdir=None):
    nc = get_program()
    in_maps = make_in_maps(feats, masks)
    res = run_bass_kernel_spmd(
        nc, in_maps, core_ids=list(range(N_CORES)),
        trace=_trace, tmpdir=_tmpdir,
    )
    out = np.concatenate([r["out"] for r in res.results], axis=0)
    out = out.astype(np.float32)
    if _trace:
        _CACHE["last_results"] = res
    return out
